# revision 34
# baseline (speedup 1.0000x reference)
"""Self-contained Trainium2 (Bass/Tile) kernel for causal multi-head
self-attention, SPMD over 8 NeuronCores.

Problem (hardcoded): B=4, T=2048, D=1024, H=16 heads, dk=64, fp32 I/O:
    q/k/v = x @ w{q,k,v} + b{q,k,v}; per-head causal softmax; y @ wo + bo.

Sharding: core c handles batch b = c // 2 and head-group g = c % 2 (8 of
16 heads; wq/wk/wv column-sharded, wo row-sharded). Each core produces a
partial [T, D] output; the host sums the two partials per batch (the
tensor-parallel reduce), adds bo, and stacks batches.

Per-core pipeline, all bf16 matmuls (PE streams bf16 at 1 cycle/row;
rel-err budget is 2e-2, bf16 lands ~4e-3):
  One pass over x: per 512-wide tq chunk ("sub"), project q/k for all 4
  head-pairs and v for all 8 heads from shared x tiles, then emit the
  causal attention chunk n == sub for every pair (kT as the stationary
  operand so scores land [tk, tq] and no transposes are needed), then
  the output projection for the 4 finished tq blocks. This keeps dense
  matmul work available at every point so the PE stays HAM-warm.

  Scores are computed unclipped in [128, 2, 512] PSUM groups (2 banks)
  so a single ScalarE exp covers 2 tk-blocks (amortizes the ~170-cycle
  ACT overhead); causal masking multiplies 0/1 bf16 masks over the 2
  diagonal groups per chunk only. v carries an appended ones column so
  softmax denominators fall out of the AV accumulation; denominators
  for all 8 heads of a chunk-set are gathered into one [8, 512] tile
  and inverted with a single DVE reciprocal (a [1,512] reciprocal runs
  on one DVE lane at 8 cycles/elem -- batching is 4x fewer of those).

kernel() self-checks a 256-query probe against a host fp32 reference
and transparently re-runs with exact fp32 matmuls if the probe misses
tolerance (BASS_ATTN_TOL, default 1e-2; harness gate is 2e-2).
"""

from contextlib import ExitStack

import numpy as np

B, T_GLOBAL, D_GLOBAL, H, DK = 4, 2048, 1024, 16, 64
HL = H // 2              # heads per core
GW = HL * DK             # 512, per-core projection width
N_CORES = 8

_NC_CACHE = {}
LAST_EXEC_TIME_NS = None
LAST_RESULT = None


def _build_nc(mm_name):
    import concourse.mybir as mybir
    import concourse.tile as tile
    from concourse import bacc
    F32 = mybir.dt.float32
    BF16 = mybir.dt.bfloat16
    FP8 = mybir.dt.float8e4
    AF = mybir.ActivationFunctionType
    fp8qk = mm_name == "fp8qk"   # q/k projections in fp8 DoubleRow
    mm_dt = {"f32r": mybir.dt.float32r, "bf16": BF16, "fp8qk": BF16,
             "f32": F32}[mm_name]
    T, D = T_GLOBAL, D_GLOBAL
    GW = HL * DK            # 512
    KS = D // 128           # 8  k-slices of the contraction dim
    TB = T // 128           # 16 t-blocks
    NSUB = T // 512         # 4  tq chunks of 512
    PAIRS = HL // 2         # 4
    HL2 = HL // 2
    scale = 1.0 / float(np.sqrt(DK))
    if fp8qk:
        scale /= 256.0    # q,k both carry a x16 host prescale
    MMDT = mm_dt
    nc = bacc.Bacc("TRN2", target_bir_lowering=False, debug=False)

    QKDT = FP8 if fp8qk else MMDT
    # ---- DRAM I/O (per-core shards, host-rearranged for contiguous DMA) ----
    xT = nc.dram_tensor("xT", [128, KS, T], MMDT, kind="ExternalInput")
    if fp8qk:
        xT8 = nc.dram_tensor("xT8", [128, KS, T], FP8, kind="ExternalInput")
    wq = nc.dram_tensor("wq", [128, KS, GW], QKDT, kind="ExternalInput")
    wk = nc.dram_tensor("wk", [128, KS, GW], QKDT, kind="ExternalInput")
    wv = nc.dram_tensor("wv", [128, KS, GW], MMDT, kind="ExternalInput")
    bq = nc.dram_tensor("bq", [128, PAIRS], F32, kind="ExternalInput")
    bk = nc.dram_tensor("bk", [128, PAIRS], F32, kind="ExternalInput")
    bv = nc.dram_tensor("bv", [1, GW], MMDT, kind="ExternalInput")
    wo = nc.dram_tensor("wo", [128, HL2, D], MMDT, kind="ExternalInput")
    out = nc.dram_tensor("out", [T, D], F32 if mm_name == "f32" else MMDT,
                         kind="ExternalOutput")

    def mm(out_ap, lhsT, rhs, start, stop):
        nc.tensor.matmul(out_ap, lhsT, rhs, start=start, stop=stop)

    with ExitStack() as top:
        tc = top.enter_context(tile.TileContext(nc))
        # PSUM budget (8 banks): psQ 2x1 (proj/out-proj) + psS 2x2
        # (score groups) + psY 2x1 (AV accumulators) = 8.
        psQ = top.enter_context(tc.tile_pool(name="psQ", bufs=2, space="PSUM"))
        psS = top.enter_context(tc.tile_pool(name="psS", bufs=4, space="PSUM"))
        psY = top.enter_context(tc.tile_pool(name="psY", bufs=2, space="PSUM"))
        const = top.enter_context(tc.tile_pool(name="const", bufs=1))
        wp = top.enter_context(tc.tile_pool(name="wp", bufs=1))
        vp = top.enter_context(tc.tile_pool(name="vp", bufs=1))
        small = mm_name != "bf16"   # fp32 fallback: fit in SBUF, speed moot
        xs = top.enter_context(tc.tile_pool(name="xs", bufs=1 if small else 2))
        qk = top.enter_context(tc.tile_pool(name="qk", bufs=2 * PAIRS))
        yp = top.enter_context(tc.tile_pool(name="yp", bufs=PAIRS))
        pp = top.enter_context(tc.tile_pool(name="pp", bufs=2 if small else 4))
        sm = top.enter_context(tc.tile_pool(name="sm", bufs=2))
        rbp = top.enter_context(tc.tile_pool(name="rbp", bufs=2 if small
                                             else 4))
        yw = top.enter_context(tc.tile_pool(name="yw", bufs=2 if small
                                            else 4))

        # ---- constants ----
        bv_row = const.tile([1, GW], MMDT, tag="bv_row", name="bv_row")
        nc.sync.dma_start(bv_row[:], bv[:])
        bv_bc = const.tile([128, GW], MMDT, tag="bv_bc", name="bv_bc")
        nc.gpsimd.partition_broadcast(bv_bc[:].bitcast(F32),
                                      bv_row[:].bitcast(F32))
        bq_sb = const.tile([128, PAIRS], F32, tag="bq", name="bq")
        nc.sync.dma_start(bq_sb[:], bq[:])
        bk_sb = const.tile([128, PAIRS], F32, tag="bk", name="bk")
        nc.sync.dma_start(bk_sb[:], bk[:])
        # 4 causal 0/1 mask variants [128, 512]: keep where tq >= tk + 128*i
        m01 = const.tile([128, 4, 512], BF16, tag="m01", name="m01")
        nc.gpsimd.memset(m01[:], 1.0)
        for i in range(4):
            nc.gpsimd.affine_select(
                out=m01[:, i, :], in_=m01[:, i, :],
                compare_op=mybir.AluOpType.is_ge,
                fill=0.0, base=-128 * i,
                pattern=[[1, 512]], channel_multiplier=-1,
            )

        # weights; per-k-slice loads ordered so the first projections'
        # inputs land first (q/k weights, then v, then wo for sub 1)
        wq_sb = wp.tile([128, KS, GW], QKDT, tag="wq", name="wq")
        wk_sb = wp.tile([128, KS, GW], QKDT, tag="wk", name="wk")
        wv_sb = wp.tile([128, KS, GW], MMDT, tag="wv", name="wv")
        xh0 = xs.tile([128, KS, 512], MMDT, tag="x", name="x")
        xh80 = None
        if fp8qk:
            xh80 = xs.tile([128, KS, 512], FP8, tag="x8", name="x8")
        nc.sync.dma_start(wq_sb[:, 0, :], wq[:, 0, :])
        nc.sync.dma_start(wk_sb[:, 0, :], wk[:, 0, :])
        # sub-0 activations jump the weight queue so the first
        # projections are not stuck behind ~4MB of weight DMA
        if fp8qk:
            nc.sync.dma_start(xh80[:, 0:2, :], xT8[:, 0:2, 0:512])
        nc.sync.dma_start(xh0[:, 0:2, :], xT[:, 0:2, 0:512])
        for k_ in range(1, KS):
            nc.sync.dma_start(wq_sb[:, k_, :], wq[:, k_, :])
            nc.sync.dma_start(wk_sb[:, k_, :], wk[:, k_, :])
        if fp8qk:
            nc.sync.dma_start(xh80[:, 2:KS, :], xT8[:, 2:KS, 0:512])
        nc.sync.dma_start(xh0[:, 2:KS, :], xT[:, 2:KS, 0:512])
        for k_ in range(KS):
            nc.sync.dma_start(wv_sb[:, k_, :], wv[:, k_, :])
        wo_sb = wp.tile([128, HL2, D], MMDT, tag="wo", name="wo_sb")
        for hp_ in range(HL2):
            nc.sync.dma_start(wo_sb[:, hp_, :], wo[:, hp_, :])

        # v_aug[:, tb, h, 0:DK] = v rows; [..., DK] = 1.0 (sums column)
        v_aug = vp.tile([128, TB, HL, DK + 1], MMDT, tag="v_aug", name="v_aug")
        nc.gpsimd.memset(v_aug[:, :, :, DK:DK + 1], 1.0)

        qts = {}
        kts = {}
        yT_rd = {}
        for pr in range(PAIRS):
            qts[pr] = qk.tile([128, T], MMDT, tag="qT", name="qT")
            kts[pr] = qk.tile([128, T], MMDT, tag="qT", name="kT")
            yT_rd[pr] = yp.tile([128, T], MMDT, tag="yt", name="yT_rd")

        # Output projection for sub s (emitted at the top of sub s+1 so
        # its PSUM slot request never blocks the next sub's projections
        # while the normalize chain drains).
        def emit_out_sub(s):
            for t8 in range(4):
                tb = s * 4 + t8
                for c2 in range(D // 512):
                    ops = psQ.tile([128, 512], F32, tag="pq", name="ops")
                    for hp in range(HL2):
                        mm(ops[:],
                           yT_rd[hp][:, tb * 128:(tb + 1) * 128],
                           wo_sb[:, hp, c2 * 512:(c2 + 1) * 512],
                           start=(hp == 0), stop=(hp == HL2 - 1))
                    osb = yw.tile([128, 512], MMDT, tag="osb", name="osb")
                    nc.vector.tensor_copy(osb[:], ops[:])
                    nc.sync.dma_start(
                        out[tb * 128:(tb + 1) * 128,
                            c2 * 512:(c2 + 1) * 512], osb[:])

        def qk_proj(sub, xhs, pr):
            col = sub * 512
            qps = psQ.tile([128, 512], F32, tag="pq", name="qps")
            kps = psQ.tile([128, 512], F32, tag="pq", name="kps")
            for k in range(KS):
                mm(qps[:], wq_sb[:, k, pr * 128:(pr + 1) * 128],
                   xhs[:, k, :], start=(k == 0), stop=(k == KS - 1))
                mm(kps[:], wk_sb[:, k, pr * 128:(pr + 1) * 128],
                   xhs[:, k, :], start=(k == 0), stop=(k == KS - 1))
            nc.vector.tensor_scalar_add(
                qts[pr][:, col:col + 512], qps[:], bq_sb[:, pr:pr + 1])
            nc.vector.tensor_scalar_add(
                kts[pr][:, col:col + 512], kps[:], bk_sb[:, pr:pr + 1])

        def v_proj(sub, xhs, t8):
            vps = psQ.tile([128, GW], F32, tag="pq", name="vps")
            for k in range(KS):
                mm(vps[:], xhs[:, k, t8 * 128:(t8 + 1) * 128],
                   wv_sb[:, k, :], start=(k == 0), stop=(k == KS - 1))
            tb = sub * 4 + t8
            nc.vector.tensor_add(
                v_aug[:, tb, :, 0:DK],
                vps[:].rearrange("p (h d) -> p h d", h=HL),
                bv_bc[:].rearrange("p (h d) -> p h d", h=HL))

        # prologue: sub 0 projections (x0 jumped the weight DMA queue)
        for pr in range(PAIRS):
            qk_proj(0, xh0, pr)
        for t8 in range(4):
            v_proj(0, xh0, t8)

        xh_next = None
        for sub in range(NSUB):
            col = sub * 512
            if sub + 1 < NSUB:
                xh_next = xs.tile([128, KS, 512], MMDT, tag="x", name="x")
                nc.sync.dma_start(xh_next[:],
                                  xT[:, :, (sub + 1) * 512:(sub + 2) * 512])
            if sub > 0:
                emit_out_sub(sub - 1)

            # ---- attention chunk n == sub for every pair, with the
            # next sub's projections threaded between pairs so the PE
            # always has dense independent work while exp ACTs drain
            # (keeps the HAM clock gate warm) ----
            jmax = 4 * sub + 3
            for pr in range(PAIRS):
                yy = [psY.tile([DK + 1, 512], F32, tag="y", name="yy")
                      for _ in range(2)]

                def emit_av(j, pts, lo):
                    for h in range(2):
                        hl = pr * 2 + h
                        mm(yy[h][:, lo:512], v_aug[:, j, hl, :],
                           pts[h][:, lo:512],
                           start=(j == 0), stop=(j == jmax))

                # Per-j score tiles, clipped to the causal region
                # [lo:512] everywhere (PSUM outside [lo:512] is never
                # read so its stale contents are harmless). AV for j is
                # emitted one j behind its exp so the PE always has
                # ready score work ahead of the exp-dependent AVs.
                prev = None
                for j in range(jmax + 1):
                    di = j - (jmax - 3)
                    lo = 128 * di if di > 0 else 0
                    sg = [psS.tile([128, 512], F32, tag="s", name="sg")
                          for _ in range(2)]
                    for h in range(2):
                        po = h * DK
                        mm(sg[h][:, lo:512],
                           kts[pr][po:po + DK, j * 128:(j + 1) * 128],
                           qts[pr][po:po + DK, col + lo:col + 512],
                           start=True, stop=True)
                    pts = []
                    for h in range(2):
                        pt = pp.tile([128, 512], MMDT, tag="pt", name="pt")
                        nc.scalar.activation(pt[:, lo:512], sg[h][:, lo:512],
                                             AF.Exp, scale=scale)
                        if di >= 0:
                            nc.vector.tensor_mul(
                                pt[:, lo:512], pt[:, lo:512],
                                m01[:, di, lo:512])
                        pts.append(pt)
                    if prev is not None:
                        emit_av(*prev)
                    prev = (j, pts, lo)
                emit_av(*prev)
                # Evict the accumulator to SBUF immediately so the PSUM
                # bank frees without waiting on the normalize chain; then
                # normalize from SBUF (reciprocal tolerates the cross-
                # partition read; tensor_tensor does not, so the final
                # partition remap into yT_rd rides a SBUF->SBUF DMA).
                for h in range(2):
                    yu = yw.tile([DK + 1, 512], MMDT, tag="yu", name="yu")
                    nc.vector.tensor_copy(yu[:], yy[h][:])
                    rs = sm.tile([1, 512], MMDT, tag="rs", name="rs")
                    with nc.allow_low_precision("softmax 1/den in bf16 is "
                                                "within output tolerance"):
                        nc.vector.reciprocal(rs[0:1, :], yu[DK:DK + 1, :])
                    rb = rbp.tile([DK, 512], MMDT, tag="rb", name="rb")
                    nc.gpsimd.partition_broadcast(
                        rb[:].bitcast(F32), rs[0:1, :].bitcast(F32))
                    yn = yw.tile([DK, 512], MMDT, tag="yn", name="yn")
                    nc.vector.tensor_mul(yn[:], yu[0:DK, :], rb[:])
                    nc.sync.dma_start(
                        yT_rd[pr][h * DK:(h + 1) * DK, col:col + 512],
                        yn[:])
                if sub + 1 < NSUB:
                    qk_proj(sub + 1, xh_next, pr)

            if sub + 1 < NSUB:
                for t8 in range(4):
                    v_proj(sub + 1, xh_next, t8)

        emit_out_sub(NSUB - 1)

    nc.compile()
    return nc


def _get_nc(mm_name):
    nc = _NC_CACHE.get(mm_name)
    if nc is None:
        nc = _NC_CACHE[mm_name] = _build_nc(mm_name)
    return nc


def _shard_inputs(x, wq, bq, wk, bk, wv, bv, wo, bo):
    T, D = T_GLOBAL, D_GLOBAL
    KS = D // 128
    PAIRS = HL // 2
    in_maps = []
    for c in range(N_CORES):
        b, g = c // 2, c % 2
        cols = slice(g * GW, (g + 1) * GW)
        xTr = np.ascontiguousarray(
            x[b].T.reshape(KS, 128, T).transpose(1, 0, 2))
        wq_c = np.ascontiguousarray(
            wq[:, cols].reshape(KS, 128, GW).transpose(1, 0, 2))
        wk_c = np.ascontiguousarray(
            wk[:, cols].reshape(KS, 128, GW).transpose(1, 0, 2))
        wv_c = np.ascontiguousarray(
            wv[:, cols].reshape(KS, 128, GW).transpose(1, 0, 2))
        bq_c = np.ascontiguousarray(bq[cols].reshape(PAIRS, 128).T)
        bk_c = np.ascontiguousarray(bk[cols].reshape(PAIRS, 128).T)
        bv_c = np.ascontiguousarray(bv[cols].reshape(1, GW))
        wo_c = np.ascontiguousarray(
            wo[cols, :].reshape(HL // 2, 2, DK, D)
            .transpose(1, 2, 0, 3).reshape(128, HL // 2, D))
        in_maps.append(dict(
            xT=xTr, wq=wq_c, wk=wk_c, wv=wv_c, bq=bq_c, bk=bk_c, bv=bv_c,
            wo=wo_c))
    return in_maps


def _probe_reference(x, wq, bq, wk, bk, wv, bv, wo, bo, nq=256):
    """fp32 host reference for output rows [0:nq] of batch 0 (causal:
    keys beyond nq never contribute)."""
    D = D_GLOBAL
    xs_ = x[0][:nq].astype(np.float32)
    q = xs_ @ wq + bq
    k = xs_ @ wk + bk
    v = xs_ @ wv + bv
    outp = np.zeros((nq, D), dtype=np.float32)
    causal = np.tril(np.ones((nq, nq), dtype=bool))
    for h in range(H):
        sl = slice(h * DK, (h + 1) * DK)
        s = (q[:, sl] @ k[:, sl].T) / np.float32(np.sqrt(DK))
        s = np.where(causal, s, -np.inf)
        p = np.exp(s - s.max(axis=1, keepdims=True))
        p /= p.sum(axis=1, keepdims=True)
        outp += (p @ v[:, sl]) @ wo[sl, :]
    return outp + bo


def _cast_in_map(in_map, mm_name):
    if mm_name == "f32":
        return in_map
    import ml_dtypes
    bf16 = np.dtype(ml_dtypes.bfloat16)
    out = {}
    for k, v in in_map.items():
        out[k] = v.astype(bf16) if k in ("xT", "wq", "wk", "wv", "bv", "wo") \
            else v
    if mm_name == "fp8qk":
        f8 = np.dtype(ml_dtypes.float8_e4m3)
        out["xT8"] = in_map["xT"].astype(f8)
        out["wq"] = (in_map["wq"] * np.float32(16)).astype(f8)
        out["wk"] = (in_map["wk"] * np.float32(16)).astype(f8)
        out["bq"] = in_map["bq"] * np.float32(16)
        out["bk"] = in_map["bk"] * np.float32(16)
    return out


def kernel(x, wq, bq, wk, bk, wv, bv, wo, bo):
    global LAST_EXEC_TIME_NS, LAST_RESULT
    import os
    from concourse.bass_utils import run_bass_kernel_spmd
    trace = bool(os.environ.get("BASS_ATTN_TRACE"))
    tol = float(os.environ.get("BASS_ATTN_TOL", "1e-2"))

    args = [np.ascontiguousarray(np.asarray(a, dtype=np.float32))
            for a in (x, wq, bq, wk, bk, wv, bv, wo, bo)]
    x, wq, bq, wk, bk, wv, bv, wo, bo = args
    in_maps = _shard_inputs(x, wq, bq, wk, bk, wv, bv, wo, bo)

    probe = _probe_reference(x, wq, bq, wk, bk, wv, bv, wo, bo)
    pden = float(np.abs(probe).max())

    def gather(res):
        T, D = T_GLOBAL, D_GLOBAL
        outf = np.empty((B, T, D), dtype=np.float32)
        for b in range(B):
            outf[b] = (res.results[2 * b]["out"].astype(np.float32)
                       + res.results[2 * b + 1]["out"].astype(np.float32)
                       + bo)
        return outf

    out_full = None
    for mm_name in ("bf16", "f32"):
        try:
            res = run_bass_kernel_spmd(
                _get_nc(mm_name),
                [_cast_in_map(m, mm_name) for m in in_maps],
                list(range(N_CORES)), trace=trace)
        except Exception:
            if mm_name == "f32":
                if out_full is not None:
                    return out_full     # best effort: keep bf16 result
                raise
            continue
        out_full = gather(res)
        LAST_EXEC_TIME_NS = res.exec_time_ns
        LAST_RESULT = res
        rel = float(np.abs(out_full[0][:probe.shape[0]] - probe).max()) / pden
        if np.isfinite(rel) and rel < tol:
            break
        # bf16 precision insufficient (unexpected) -> exact fp32 fallback
    return out_full


# revision 36
# speedup vs baseline: 1.0064x; 1.0064x over previous
"""Self-contained Trainium2 (Bass/Tile) kernel for causal multi-head
self-attention, SPMD over 8 NeuronCores.

Problem (hardcoded): B=4, T=2048, D=1024, H=16 heads, dk=64, fp32 I/O:
    q/k/v = x @ w{q,k,v} + b{q,k,v}; per-head causal softmax; y @ wo + bo.

Sharding: core c handles batch b = c // 2 and head-group g = c % 2 (8 of
16 heads; wq/wk/wv column-sharded, wo row-sharded). Each core produces a
partial [T, D] output; the host sums the two partials per batch (the
tensor-parallel reduce), adds bo, and stacks batches.

Per-core pipeline, all bf16 matmuls (PE streams bf16 at 1 cycle/row;
rel-err budget is 2e-2, bf16 lands ~4e-3):
  One pass over x: per 512-wide tq chunk ("sub"), project q/k for all 4
  head-pairs and v for all 8 heads from shared x tiles, then emit the
  causal attention chunk n == sub for every pair (kT as the stationary
  operand so scores land [tk, tq] and no transposes are needed), then
  the output projection for the 4 finished tq blocks. This keeps dense
  matmul work available at every point so the PE stays HAM-warm.

  Scores are computed unclipped in [128, 2, 512] PSUM groups (2 banks)
  so a single ScalarE exp covers 2 tk-blocks (amortizes the ~170-cycle
  ACT overhead); causal masking multiplies 0/1 bf16 masks over the 2
  diagonal groups per chunk only. v carries an appended ones column so
  softmax denominators fall out of the AV accumulation; denominators
  for all 8 heads of a chunk-set are gathered into one [8, 512] tile
  and inverted with a single DVE reciprocal (a [1,512] reciprocal runs
  on one DVE lane at 8 cycles/elem -- batching is 4x fewer of those).

kernel() self-checks a 256-query probe against a host fp32 reference
and transparently re-runs with exact fp32 matmuls if the probe misses
tolerance (BASS_ATTN_TOL, default 1e-2; harness gate is 2e-2).
"""

from contextlib import ExitStack

import numpy as np

B, T_GLOBAL, D_GLOBAL, H, DK = 4, 2048, 1024, 16, 64
HL = H // 2              # heads per core
GW = HL * DK             # 512, per-core projection width
N_CORES = 8

_NC_CACHE = {}
LAST_EXEC_TIME_NS = None
LAST_RESULT = None


def _build_nc(mm_name):
    import concourse.mybir as mybir
    import concourse.tile as tile
    from concourse import bacc
    F32 = mybir.dt.float32
    BF16 = mybir.dt.bfloat16
    FP8 = mybir.dt.float8e4
    AF = mybir.ActivationFunctionType
    fp8qk = mm_name == "fp8qk"   # q/k projections in fp8 DoubleRow
    mm_dt = {"f32r": mybir.dt.float32r, "bf16": BF16, "fp8qk": BF16,
             "f32": F32}[mm_name]
    T, D = T_GLOBAL, D_GLOBAL
    GW = HL * DK            # 512
    KS = D // 128           # 8  k-slices of the contraction dim
    TB = T // 128           # 16 t-blocks
    NSUB = T // 512         # 4  tq chunks of 512
    PAIRS = HL // 2         # 4
    HL2 = HL // 2
    scale = 1.0 / float(np.sqrt(DK))
    if fp8qk:
        scale /= 256.0    # q,k both carry a x16 host prescale
    MMDT = mm_dt
    nc = bacc.Bacc("TRN2", target_bir_lowering=False, debug=False)

    QKDT = FP8 if fp8qk else MMDT
    # ---- DRAM I/O (per-core shards, host-rearranged for contiguous DMA) ----
    xT = nc.dram_tensor("xT", [128, KS, T], MMDT, kind="ExternalInput")
    if fp8qk:
        xT8 = nc.dram_tensor("xT8", [128, KS, T], FP8, kind="ExternalInput")
    wq = nc.dram_tensor("wq", [128, KS, GW], QKDT, kind="ExternalInput")
    wk = nc.dram_tensor("wk", [128, KS, GW], QKDT, kind="ExternalInput")
    wv = nc.dram_tensor("wv", [128, KS, GW], MMDT, kind="ExternalInput")
    bq = nc.dram_tensor("bq", [128, PAIRS], F32, kind="ExternalInput")
    bk = nc.dram_tensor("bk", [128, PAIRS], F32, kind="ExternalInput")
    bv = nc.dram_tensor("bv", [1, GW], MMDT, kind="ExternalInput")
    wo = nc.dram_tensor("wo", [128, HL2, D], MMDT, kind="ExternalInput")
    out = nc.dram_tensor("out", [T, D], F32 if mm_name == "f32" else MMDT,
                         kind="ExternalOutput")

    def mm(out_ap, lhsT, rhs, start, stop):
        nc.tensor.matmul(out_ap, lhsT, rhs, start=start, stop=stop)

    with ExitStack() as top:
        tc = top.enter_context(tile.TileContext(nc))
        # PSUM budget (8 banks): psQ 2x1 (proj/out-proj) + psS 2x2
        # (score groups) + psY 2x1 (AV accumulators) = 8.
        psQ = top.enter_context(tc.tile_pool(name="psQ", bufs=2, space="PSUM"))
        psS = top.enter_context(tc.tile_pool(name="psS", bufs=4, space="PSUM"))
        psY = top.enter_context(tc.tile_pool(name="psY", bufs=2, space="PSUM"))
        const = top.enter_context(tc.tile_pool(name="const", bufs=1))
        wp = top.enter_context(tc.tile_pool(name="wp", bufs=1))
        vp = top.enter_context(tc.tile_pool(name="vp", bufs=1))
        small = mm_name != "bf16"   # fp32 fallback: fit in SBUF, speed moot
        xs = top.enter_context(tc.tile_pool(name="xs", bufs=1 if small else 2))
        qk = top.enter_context(tc.tile_pool(name="qk", bufs=2 * PAIRS))
        yp = top.enter_context(tc.tile_pool(name="yp", bufs=PAIRS))
        pp = top.enter_context(tc.tile_pool(name="pp", bufs=2 if small else 4))
        sm = top.enter_context(tc.tile_pool(name="sm", bufs=2))
        rbp = top.enter_context(tc.tile_pool(name="rbp", bufs=2 if small
                                             else 4))
        yw = top.enter_context(tc.tile_pool(name="yw", bufs=2 if small
                                            else 4))

        # ---- constants ----
        bv_row = const.tile([1, GW], MMDT, tag="bv_row", name="bv_row")
        nc.sync.dma_start(bv_row[:], bv[:])
        bv_bc = const.tile([128, GW], MMDT, tag="bv_bc", name="bv_bc")
        nc.gpsimd.partition_broadcast(bv_bc[:].bitcast(F32),
                                      bv_row[:].bitcast(F32))
        bq_sb = const.tile([128, PAIRS], F32, tag="bq", name="bq")
        nc.sync.dma_start(bq_sb[:], bq[:])
        bk_sb = const.tile([128, PAIRS], F32, tag="bk", name="bk")
        nc.sync.dma_start(bk_sb[:], bk[:])
        # 4 causal 0/1 mask variants [128, 512]: keep where tq >= tk + 128*i
        m01 = const.tile([128, 4, 512], BF16, tag="m01", name="m01")
        nc.gpsimd.memset(m01[:], 1.0)
        for i in range(4):
            nc.gpsimd.affine_select(
                out=m01[:, i, :], in_=m01[:, i, :],
                compare_op=mybir.AluOpType.is_ge,
                fill=0.0, base=-128 * i,
                pattern=[[1, 512]], channel_multiplier=-1,
            )

        if mm_name == "bf16":
            # PE warm-up: throwaway matmuls over the mask constants fill
            # the initial weight/x DMA-wait window (~15us of PE idle)
            # and bring the HAM clock gate to K=8/8 before the real
            # stream starts. The output bank is never read.
            ws = psS.tile([128, 512], F32, tag="s", name="warm")
            for _ in range(32):
                mm(ws[:], m01[:, 1, 0:128], m01[:, 0, :],
                   start=True, stop=True)

        # weights; per-k-slice loads ordered so the first projections'
        # inputs land first (q/k weights, then v, then wo for sub 1)
        wq_sb = wp.tile([128, KS, GW], QKDT, tag="wq", name="wq")
        wk_sb = wp.tile([128, KS, GW], QKDT, tag="wk", name="wk")
        wv_sb = wp.tile([128, KS, GW], MMDT, tag="wv", name="wv")
        xh0 = xs.tile([128, KS, 512], MMDT, tag="x", name="x")
        xh80 = None
        if fp8qk:
            xh80 = xs.tile([128, KS, 512], FP8, tag="x8", name="x8")
        nc.sync.dma_start(wq_sb[:, 0, :], wq[:, 0, :])
        nc.sync.dma_start(wk_sb[:, 0, :], wk[:, 0, :])
        # sub-0 activations jump the weight queue so the first
        # projections are not stuck behind ~4MB of weight DMA
        if fp8qk:
            nc.sync.dma_start(xh80[:, 0:2, :], xT8[:, 0:2, 0:512])
        nc.sync.dma_start(xh0[:, 0:2, :], xT[:, 0:2, 0:512])
        for k_ in range(1, KS):
            nc.sync.dma_start(wq_sb[:, k_, :], wq[:, k_, :])
            nc.sync.dma_start(wk_sb[:, k_, :], wk[:, k_, :])
        if fp8qk:
            nc.sync.dma_start(xh80[:, 2:KS, :], xT8[:, 2:KS, 0:512])
        nc.sync.dma_start(xh0[:, 2:KS, :], xT[:, 2:KS, 0:512])
        for k_ in range(KS):
            nc.sync.dma_start(wv_sb[:, k_, :], wv[:, k_, :])
        wo_sb = wp.tile([128, HL2, D], MMDT, tag="wo", name="wo_sb")
        for hp_ in range(HL2):
            nc.sync.dma_start(wo_sb[:, hp_, :], wo[:, hp_, :])

        # v_aug[:, tb, h, 0:DK] = v rows; [..., DK] = 1.0 (sums column)
        v_aug = vp.tile([128, TB, HL, DK + 1], MMDT, tag="v_aug", name="v_aug")
        nc.gpsimd.memset(v_aug[:, :, :, DK:DK + 1], 1.0)

        qts = {}
        kts = {}
        yT_rd = {}
        for pr in range(PAIRS):
            qts[pr] = qk.tile([128, T], MMDT, tag="qT", name="qT")
            kts[pr] = qk.tile([128, T], MMDT, tag="qT", name="kT")
            yT_rd[pr] = yp.tile([128, T], MMDT, tag="yt", name="yT_rd")

        # Output projection for sub s (emitted at the top of sub s+1 so
        # its PSUM slot request never blocks the next sub's projections
        # while the normalize chain drains).
        def emit_out_sub(s):
            for t8 in range(4):
                tb = s * 4 + t8
                for c2 in range(D // 512):
                    ops = psQ.tile([128, 512], F32, tag="pq", name="ops")
                    for hp in range(HL2):
                        mm(ops[:],
                           yT_rd[hp][:, tb * 128:(tb + 1) * 128],
                           wo_sb[:, hp, c2 * 512:(c2 + 1) * 512],
                           start=(hp == 0), stop=(hp == HL2 - 1))
                    osb = yw.tile([128, 512], MMDT, tag="osb", name="osb")
                    nc.vector.tensor_copy(osb[:], ops[:])
                    nc.sync.dma_start(
                        out[tb * 128:(tb + 1) * 128,
                            c2 * 512:(c2 + 1) * 512], osb[:])

        def qk_proj(sub, xhs, pr):
            col = sub * 512
            qps = psQ.tile([128, 512], F32, tag="pq", name="qps")
            kps = psQ.tile([128, 512], F32, tag="pq", name="kps")
            for k in range(KS):
                mm(qps[:], wq_sb[:, k, pr * 128:(pr + 1) * 128],
                   xhs[:, k, :], start=(k == 0), stop=(k == KS - 1))
                mm(kps[:], wk_sb[:, k, pr * 128:(pr + 1) * 128],
                   xhs[:, k, :], start=(k == 0), stop=(k == KS - 1))
            nc.vector.tensor_scalar_add(
                qts[pr][:, col:col + 512], qps[:], bq_sb[:, pr:pr + 1])
            nc.vector.tensor_scalar_add(
                kts[pr][:, col:col + 512], kps[:], bk_sb[:, pr:pr + 1])

        def v_proj(sub, xhs, t8):
            vps = psQ.tile([128, GW], F32, tag="pq", name="vps")
            for k in range(KS):
                mm(vps[:], xhs[:, k, t8 * 128:(t8 + 1) * 128],
                   wv_sb[:, k, :], start=(k == 0), stop=(k == KS - 1))
            tb = sub * 4 + t8
            nc.vector.tensor_add(
                v_aug[:, tb, :, 0:DK],
                vps[:].rearrange("p (h d) -> p h d", h=HL),
                bv_bc[:].rearrange("p (h d) -> p h d", h=HL))

        # prologue: sub 0 projections (x0 jumped the weight DMA queue)
        for pr in range(PAIRS):
            qk_proj(0, xh0, pr)
        for t8 in range(4):
            v_proj(0, xh0, t8)

        xh_next = None
        for sub in range(NSUB):
            col = sub * 512
            if sub + 1 < NSUB:
                xh_next = xs.tile([128, KS, 512], MMDT, tag="x", name="x")
                nc.sync.dma_start(xh_next[:],
                                  xT[:, :, (sub + 1) * 512:(sub + 2) * 512])
            if sub > 0:
                emit_out_sub(sub - 1)

            # ---- attention chunk n == sub for every pair, with the
            # next sub's projections threaded between pairs so the PE
            # always has dense independent work while exp ACTs drain
            # (keeps the HAM clock gate warm) ----
            jmax = 4 * sub + 3
            for pr in range(PAIRS):
                yy = [psY.tile([DK + 1, 512], F32, tag="y", name="yy")
                      for _ in range(2)]

                def emit_av(j, pts, lo):
                    for h in range(2):
                        hl = pr * 2 + h
                        mm(yy[h][:, lo:512], v_aug[:, j, hl, :],
                           pts[h][:, lo:512],
                           start=(j == 0), stop=(j == jmax))

                # Per-j score tiles, clipped to the causal region
                # [lo:512] everywhere (PSUM outside [lo:512] is never
                # read so its stale contents are harmless). AV for j is
                # emitted one j behind its exp so the PE always has
                # ready score work ahead of the exp-dependent AVs.
                prev = None
                for j in range(jmax + 1):
                    di = j - (jmax - 3)
                    lo = 128 * di if di > 0 else 0
                    sg = [psS.tile([128, 512], F32, tag="s", name="sg")
                          for _ in range(2)]
                    for h in range(2):
                        po = h * DK
                        mm(sg[h][:, lo:512],
                           kts[pr][po:po + DK, j * 128:(j + 1) * 128],
                           qts[pr][po:po + DK, col + lo:col + 512],
                           start=True, stop=True)
                    pts = []
                    for h in range(2):
                        pt = pp.tile([128, 512], MMDT, tag="pt", name="pt")
                        nc.scalar.activation(pt[:, lo:512], sg[h][:, lo:512],
                                             AF.Exp, scale=scale)
                        if di >= 0:
                            nc.vector.tensor_mul(
                                pt[:, lo:512], pt[:, lo:512],
                                m01[:, di, lo:512])
                        pts.append(pt)
                    if prev is not None:
                        emit_av(*prev)
                    prev = (j, pts, lo)
                emit_av(*prev)
                # Evict the accumulator to SBUF immediately so the PSUM
                # bank frees without waiting on the normalize chain; then
                # normalize from SBUF (reciprocal tolerates the cross-
                # partition read; tensor_tensor does not, so the final
                # partition remap into yT_rd rides a SBUF->SBUF DMA).
                for h in range(2):
                    yu = yw.tile([DK + 1, 512], MMDT, tag="yu", name="yu")
                    # ScalarE eviction: frees the PSUM slot for the next
                    # pair's AV without queueing behind the DVE backlog
                    nc.scalar.copy(yu[:], yy[h][:])
                    rs = sm.tile([1, 512], MMDT, tag="rs", name="rs")
                    with nc.allow_low_precision("softmax 1/den in bf16 is "
                                                "within output tolerance"):
                        nc.vector.reciprocal(rs[0:1, :], yu[DK:DK + 1, :])
                    rb = rbp.tile([DK, 512], MMDT, tag="rb", name="rb")
                    nc.gpsimd.partition_broadcast(
                        rb[:].bitcast(F32), rs[0:1, :].bitcast(F32))
                    yn = yw.tile([DK, 512], MMDT, tag="yn", name="yn")
                    nc.vector.tensor_mul(yn[:], yu[0:DK, :], rb[:])
                    nc.sync.dma_start(
                        yT_rd[pr][h * DK:(h + 1) * DK, col:col + 512],
                        yn[:])
                if sub + 1 < NSUB:
                    qk_proj(sub + 1, xh_next, pr)

            if sub + 1 < NSUB:
                for t8 in range(4):
                    v_proj(sub + 1, xh_next, t8)

        emit_out_sub(NSUB - 1)

    nc.compile()
    return nc


def _get_nc(mm_name):
    nc = _NC_CACHE.get(mm_name)
    if nc is None:
        nc = _NC_CACHE[mm_name] = _build_nc(mm_name)
    return nc


def _shard_inputs(x, wq, bq, wk, bk, wv, bv, wo, bo):
    T, D = T_GLOBAL, D_GLOBAL
    KS = D // 128
    PAIRS = HL // 2
    in_maps = []
    for c in range(N_CORES):
        b, g = c // 2, c % 2
        cols = slice(g * GW, (g + 1) * GW)
        xTr = np.ascontiguousarray(
            x[b].T.reshape(KS, 128, T).transpose(1, 0, 2))
        wq_c = np.ascontiguousarray(
            wq[:, cols].reshape(KS, 128, GW).transpose(1, 0, 2))
        wk_c = np.ascontiguousarray(
            wk[:, cols].reshape(KS, 128, GW).transpose(1, 0, 2))
        wv_c = np.ascontiguousarray(
            wv[:, cols].reshape(KS, 128, GW).transpose(1, 0, 2))
        bq_c = np.ascontiguousarray(bq[cols].reshape(PAIRS, 128).T)
        bk_c = np.ascontiguousarray(bk[cols].reshape(PAIRS, 128).T)
        bv_c = np.ascontiguousarray(bv[cols].reshape(1, GW))
        wo_c = np.ascontiguousarray(
            wo[cols, :].reshape(HL // 2, 2, DK, D)
            .transpose(1, 2, 0, 3).reshape(128, HL // 2, D))
        in_maps.append(dict(
            xT=xTr, wq=wq_c, wk=wk_c, wv=wv_c, bq=bq_c, bk=bk_c, bv=bv_c,
            wo=wo_c))
    return in_maps


def _probe_reference(x, wq, bq, wk, bk, wv, bv, wo, bo, nq=256):
    """fp32 host reference for output rows [0:nq] of batch 0 (causal:
    keys beyond nq never contribute)."""
    D = D_GLOBAL
    xs_ = x[0][:nq].astype(np.float32)
    q = xs_ @ wq + bq
    k = xs_ @ wk + bk
    v = xs_ @ wv + bv
    outp = np.zeros((nq, D), dtype=np.float32)
    causal = np.tril(np.ones((nq, nq), dtype=bool))
    for h in range(H):
        sl = slice(h * DK, (h + 1) * DK)
        s = (q[:, sl] @ k[:, sl].T) / np.float32(np.sqrt(DK))
        s = np.where(causal, s, -np.inf)
        p = np.exp(s - s.max(axis=1, keepdims=True))
        p /= p.sum(axis=1, keepdims=True)
        outp += (p @ v[:, sl]) @ wo[sl, :]
    return outp + bo


def _cast_in_map(in_map, mm_name):
    if mm_name == "f32":
        return in_map
    import ml_dtypes
    bf16 = np.dtype(ml_dtypes.bfloat16)
    out = {}
    for k, v in in_map.items():
        out[k] = v.astype(bf16) if k in ("xT", "wq", "wk", "wv", "bv", "wo") \
            else v
    if mm_name == "fp8qk":
        f8 = np.dtype(ml_dtypes.float8_e4m3)
        out["xT8"] = in_map["xT"].astype(f8)
        out["wq"] = (in_map["wq"] * np.float32(16)).astype(f8)
        out["wk"] = (in_map["wk"] * np.float32(16)).astype(f8)
        out["bq"] = in_map["bq"] * np.float32(16)
        out["bk"] = in_map["bk"] * np.float32(16)
    return out


def kernel(x, wq, bq, wk, bk, wv, bv, wo, bo):
    global LAST_EXEC_TIME_NS, LAST_RESULT
    import os
    from concourse.bass_utils import run_bass_kernel_spmd
    trace = bool(os.environ.get("BASS_ATTN_TRACE"))
    tol = float(os.environ.get("BASS_ATTN_TOL", "1e-2"))

    args = [np.ascontiguousarray(np.asarray(a, dtype=np.float32))
            for a in (x, wq, bq, wk, bk, wv, bv, wo, bo)]
    x, wq, bq, wk, bk, wv, bv, wo, bo = args
    in_maps = _shard_inputs(x, wq, bq, wk, bk, wv, bv, wo, bo)

    probe = _probe_reference(x, wq, bq, wk, bk, wv, bv, wo, bo)
    pden = float(np.abs(probe).max())

    def gather(res):
        T, D = T_GLOBAL, D_GLOBAL
        outf = np.empty((B, T, D), dtype=np.float32)
        for b in range(B):
            outf[b] = (res.results[2 * b]["out"].astype(np.float32)
                       + res.results[2 * b + 1]["out"].astype(np.float32)
                       + bo)
        return outf

    out_full = None
    for mm_name in ("bf16", "f32"):
        try:
            res = run_bass_kernel_spmd(
                _get_nc(mm_name),
                [_cast_in_map(m, mm_name) for m in in_maps],
                list(range(N_CORES)), trace=trace)
        except Exception:
            if mm_name == "f32":
                if out_full is not None:
                    return out_full     # best effort: keep bf16 result
                raise
            continue
        out_full = gather(res)
        LAST_EXEC_TIME_NS = res.exec_time_ns
        LAST_RESULT = res
        rel = float(np.abs(out_full[0][:probe.shape[0]] - probe).max()) / pden
        if np.isfinite(rel) and rel < tol:
            break
        # bf16 precision insufficient (unexpected) -> exact fp32 fallback
    return out_full
